# revision 15
# baseline (speedup 1.0000x reference)
"""Trainium2 Bass kernel for the 3-layer MLP encode/decode forward pass.

Computation (B = 65536):
    d_i = pinv(W_i)                       (host, negligible)
    h = lrelu(x @ W1.T)                   [B, 128]
    h = lrelu(h @ W2.T)                   [B, 64]
    h = h @ W3.T                          [B, 16]
    h = lrelu(h @ d3.T)                   [B, 64]   (folded: lrelu((d3@W3) @ h2))
    h = lrelu(h @ d2.T)                   [B, 128]
    out = h @ d1.T                        [B, 784]

Sharding: pure data-parallel — 8 cores x 8192 batch rows each; the tiny
weights (and host-side pinv) are replicated.

The kernel is HBM-bound: 2 x B x 784 elements of I/O vs ~0.4 GFLOP of
matmul per core.  All I/O and matmul operands are fp16 (fp32 PSUM
accumulation; end-to-end rel err ~6e-4, well inside the 2e-2 gate),
which halves the DMA traffic vs fp32 and doubles DVE copy throughput
for the 16-bit transpose tiles.

Per-core layout: activations are kept feature-major ([feat, batch]) so
TensorE contracts over features.  x is transposed on-chip via PE
transpose-mode.  The final layer swaps operand roles (stationary =
activation tile, moving = d1.T) so the output lands batch-major in
PSUM — no output transpose.

Pipelining: tiles are processed in pairs with the layer ladder emitted
layer-major across the pair (mm2 A, mm2 B, mm3 A, ...), so each
engine's in-order stream always has the sibling tile's work between a
matmul and the activation that consumes it.  DMA moves `dma_group`
512-row tiles per transfer (batch order inside a group is permuted;
the permutation cancels between input transposes and output writeback).
"""

import numpy as np

B = 65536
N_CORES = 8
B_LOC = B // N_CORES  # 8192
D0, D1, D2, D3 = 784, 128, 64, 16
KCH = 112          # 784 = 7 * 112 contraction chunks for layer 1
NKC = D0 // KCH    # 7
TILE = 512         # moving free dim per matmul (one fp32 PSUM bank)
SUB = 128          # batch sub-tile (partition dim of x / out tiles)
NSUB = TILE // SUB  # 4
HALF = D0 // 2     # 392

IO_DT = "float16"  # on-chip + DRAM dtype for x / weights / out


def _build_nc(b_loc=B_LOC, repeat=1, io_dt_name=IO_DT,
              in_dma_eng="sync", out_dma_eng="scalar", dma_group=2,
              xin_bufs=3, outp_bufs=3, xtp_bufs=14, acts_bufs=2,
              ocopy="split", xcopy="vector", staggered=False,
              dma_only=False, compute_only=False,
              no_xpose=False, xpose_only=False, dma_tx=False,
              act_dve=False):
    import contextlib
    import concourse.tile as tile
    from concourse import bacc, mybir

    dt16 = getattr(mybir.dt, io_dt_name)
    f32 = mybir.dt.float32
    LRELU = mybir.ActivationFunctionType.Lrelu
    COPY = mybir.ActivationFunctionType.Copy

    nc = bacc.Bacc(trn_type="TRN2", target_bir_lowering=False, debug=False,
                   num_devices=N_CORES)

    x = nc.declare_dram_parameter("x", [b_loc, D0], dt16, isOutput=False).ap()
    w1t = nc.declare_dram_parameter("w1t", [D0, D1], dt16, isOutput=False).ap()
    w2t = nc.declare_dram_parameter("w2t", [D1, D2], dt16, isOutput=False).ap()
    m3t = nc.declare_dram_parameter("m3t", [D2, D2], dt16, isOutput=False).ap()
    d2t = nc.declare_dram_parameter("d2t", [D2, D1], dt16, isOutput=False).ap()
    d1t = nc.declare_dram_parameter("d1t", [D1, D0], dt16, isOutput=False).ap()
    ident = nc.declare_dram_parameter("ident", [SUB, SUB], dt16, isOutput=False).ap()
    out = nc.declare_dram_parameter("out", [b_loc, D0], dt16, isOutput=True).ap()

    G = dma_group
    n_tiles = b_loc // TILE
    assert n_tiles % G == 0
    n_groups = n_tiles // G
    # row = grp*512*G + p*4*G + s  (4*G batch rows per partition per group)
    x_r = x.rearrange("(n p s) f -> n p (s f)", p=SUB, s=NSUB * G)
    out_r = out.rearrange("(n p s) f -> n p (s f)", p=SUB, s=NSUB * G)

    with tile.TileContext(nc, num_cores=N_CORES, pool_alloc_mode="stack") as tc:
        with (
            tc.tile_pool(name="consts", bufs=1) as consts,
            tc.tile_pool(name="xin", bufs=xin_bufs) as xin,
            tc.tile_pool(name="xtp", bufs=xtp_bufs) as xtp,
            tc.tile_pool(name="acts", bufs=acts_bufs) as acts,
            tc.tile_pool(name="outp", bufs=outp_bufs) as outp,
            tc.tile_pool(name="psT", bufs=2, space="PSUM") as psT,
            tc.tile_pool(name="psMM", bufs=2, space="PSUM") as psMM,
            tc.tile_pool(name="psO", bufs=2, space="PSUM") as psO,
        ):
            # --- constants ---
            w1t_sb = consts.tile([KCH, NKC, D1], dt16)
            nc.sync.dma_start(out=w1t_sb, in_=w1t.rearrange("(c p) m -> p c m", p=KCH))
            w2t_sb = consts.tile([D1, D2], dt16)
            nc.sync.dma_start(out=w2t_sb, in_=w2t)
            m3t_sb = consts.tile([D2, D2], dt16)
            nc.sync.dma_start(out=m3t_sb, in_=m3t)
            d2t_sb = consts.tile([D2, D1], dt16)
            nc.sync.dma_start(out=d2t_sb, in_=d2t)
            d1t_sb = consts.tile([D1, D0], dt16)
            nc.sync.dma_start(out=d1t_sb, in_=d1t)
            id_sb = consts.tile([SUB, SUB], dt16)
            nc.sync.dma_start(out=id_sb, in_=ident)

            if compute_only:
                x_const = consts.tile([SUB, NSUB * G, D0], dt16)
                nc.sync.dma_start(out=x_const, in_=x_r[0])

            xt_const = None
            if no_xpose:
                xt_const = []
                for c in range(NKC):
                    xc = consts.tile([KCH, TILE], dt16, name=f"xt_const{c}")
                    nc.sync.dma_start(out=xc, in_=x[c * KCH:(c + 1) * KCH, :TILE])
                    xt_const.append(xc)

            def front(x_sb, j):
                """Transpose subtile j to feature-major, L1 matmul + lrelu."""
                if no_xpose:
                    xt_sb = xt_const
                else:
                    xt_sb = []
                    for c in range(NKC):
                        tp = psT.tile([KCH, TILE], dt16, tag="psT")
                        for u in range(NSUB):
                            nc.tensor.transpose(
                                out=tp[:, u * SUB:(u + 1) * SUB],
                                in_=x_sb[:, j * NSUB + u, c * KCH:(c + 1) * KCH],
                                identity=id_sb,
                            )
                        xt = xtp.tile([KCH, TILE], dt16, tag="xt")
                        if xcopy == "scalar":
                            nc.scalar.activation(out=xt, in_=tp, func=COPY)
                        else:
                            nc.vector.tensor_copy(xt, tp)
                        xt_sb.append(xt)
                if xpose_only:
                    return None
                h1_ps = psMM.tile([D1, TILE], f32, tag="mm")
                for c in range(NKC):
                    nc.tensor.matmul(h1_ps, lhsT=w1t_sb[:, c, :], rhs=xt_sb[c],
                                     start=(c == 0), stop=(c == NKC - 1))
                h1_sb = acts.tile([D1, TILE], dt16, tag="h1", name="h1_sb")
                if act_dve:
                    nc.vector.tensor_copy(h1_sb, h1_ps)
                else:
                    nc.scalar.activation(out=h1_sb, in_=h1_ps, func=LRELU,
                                         alpha=0.01)
                return h1_sb

            def step(h_sb, w_sb, m, tag):
                """One ladder layer: [m, 512] = lrelu(w_sb.T @ h_sb)."""
                ps = psMM.tile([m, TILE], f32, tag="mm", name="ps")
                nc.tensor.matmul(ps, lhsT=w_sb, rhs=h_sb, start=True, stop=True)
                sb = acts.tile([m, TILE], dt16, tag=tag, name="sb")
                if act_dve:
                    nc.vector.tensor_copy(sb, ps)
                else:
                    nc.scalar.activation(out=sb, in_=ps, func=LRELU, alpha=0.01)
                return sb

            def l5(g2_sb, o_sb, j):
                """out = g2.T @ d1.T, batch-major via stationary swap."""
                for s in range(NSUB):
                    g2c = g2_sb[:, s * SUB:(s + 1) * SUB]
                    po = psO.tile([SUB, 1024], f32, tag="po")
                    nc.tensor.matmul(po[:, :HALF], lhsT=g2c, rhs=d1t_sb[:, :HALF],
                                     start=True, stop=True)
                    nc.tensor.matmul(po[:, 512:512 + HALF], lhsT=g2c,
                                     rhs=d1t_sb[:, HALF:], start=True, stop=True)
                    po_v = po.rearrange("p (b r) -> p b r", b=2)[:, :, :HALF]
                    o_v = o_sb[:, j * NSUB + s, :].rearrange("p (b r) -> p b r", b=2)
                    if ocopy == "scalar" or (ocopy == "split" and s % 2 == 0):
                        nc.scalar.activation(out=o_v, in_=po_v, func=COPY)
                    else:
                        nc.vector.tensor_copy(o_v, po_v)

            rep_ctx = (tc.For_i(0, repeat, 1, staggered_reset=staggered)
                       if repeat > 1 else contextlib.nullcontext())
            with rep_ctx:
              for g in range(n_groups):
                # --- load G*512 rows in one DMA: [128, G*4, 784] fp16 ---
                if compute_only:
                    x_sb = x_const
                elif dma_only and dma_tx:
                    x_sb = None
                else:
                    x_sb = xin.tile([SUB, NSUB * G, D0], dt16, tag="x")
                    if in_dma_eng == "alt":
                        (nc.sync if g % 2 == 0 else nc.scalar).dma_start(
                            out=x_sb, in_=x_r[g])
                    else:
                        getattr(nc, in_dma_eng).dma_start(out=x_sb, in_=x_r[g])

                o_sb = outp.tile([SUB, NSUB * G, D0], dt16, tag="o")

                if dma_only:
                    if dma_tx:
                        for c in range(6):
                            xt_g = xtp.tile([SUB, TILE * G], dt16, tag=f"xtg{c}",
                                            bufs=2)
                            nc.sync.dma_start(
                                out=xt_g,
                                in_=x[g * TILE * G:(g + 1) * TILE * G,
                                      c * SUB:(c + 1) * SUB],
                                transpose=True)
                            nc.vector.tensor_copy(
                                o_sb.rearrange("p a b -> p (a b)")
                                    [:, c * TILE * G:(c + 1) * TILE * G], xt_g)
                    else:
                        nc.vector.tensor_copy(o_sb, x_sb)
                else:
                    for pa in range(0, G, 2):
                        A, Bt = pa, pa + 1
                        h1a = front(x_sb, A)
                        h1b = front(x_sb, Bt)
                        if xpose_only:
                            continue
                        h2a = step(h1a, w2t_sb, D2, "h2")
                        h2b = step(h1b, w2t_sb, D2, "h2")
                        g3a = step(h2a, m3t_sb, D2, "g3")
                        g3b = step(h2b, m3t_sb, D2, "g3")
                        g2a = step(g3a, d2t_sb, D1, "g2")
                        g2b = step(g3b, d2t_sb, D1, "g2")
                        l5(g2a, o_sb, A)
                        l5(g2b, o_sb, Bt)
                    if xpose_only:
                        nc.vector.tensor_copy(o_sb, x_sb)

                if not compute_only:
                    getattr(nc, out_dma_eng).dma_start(out=out_r[g], in_=o_sb)

              if compute_only:
                nc.sync.dma_start(out=out_r[0], in_=o_sb)

    nc.finalize()
    return nc


def _host_weights(W1, W2, W3):
    def pinv(W):
        u, s, vh = np.linalg.svd(W.astype(np.float64), full_matrices=False)
        return (vh.T * (1.0 / s)) @ u.T

    d1, d2, d3 = pinv(W1), pinv(W2), pinv(W3)
    f = np.float16
    return {
        "w1t": np.ascontiguousarray(W1.T, dtype=f),
        "w2t": np.ascontiguousarray(W2.T, dtype=f),
        "m3t": np.ascontiguousarray((d3 @ W3.astype(np.float64)).T, dtype=f),
        "d2t": np.ascontiguousarray(d2.T, dtype=f),
        "d1t": np.ascontiguousarray(d1.T, dtype=f),
        "ident": np.eye(SUB, dtype=f),
    }


def _in_maps(x, W1, W2, W3):
    x = np.ascontiguousarray(x, dtype=np.float16)
    w = _host_weights(np.asarray(W1), np.asarray(W2), np.asarray(W3))
    return [{"x": x[i * B_LOC:(i + 1) * B_LOC], **w} for i in range(N_CORES)]


_NC_CACHE = {}


def _get_nc(key=()):
    if key not in _NC_CACHE:
        _NC_CACHE[key] = _build_nc(B_LOC)
    return _NC_CACHE[key]


def kernel(x, W1, W2, W3):
    from concourse.bass_utils import run_bass_kernel_spmd

    in_maps = _in_maps(x, W1, W2, W3)
    nc = _get_nc()
    res = run_bass_kernel_spmd(nc, in_maps, core_ids=list(range(N_CORES)))
    return np.concatenate(
        [res.results[i]["out"] for i in range(N_CORES)], axis=0
    ).astype(np.float32)
